# revision 9
# baseline (speedup 1.0000x reference)
"""BigBird attention (faithful .view-split variant) on 8 Trainium2 NeuronCores.

Sharding: the reference's `.reshape(B, H, S, hd)` head-split makes each
(batch, head) attend over a [2048, 64] row-major reshape of a 128-token
chunk's [128, 1024] projection. The 2*16 = 32 (b,h) pairs are sharded 4 per
core (batch x head parallel). The output projection is computed per-core as
a partial sum over its 4 heads (row-parallel over Wo), partials are summed
on the host.

Per core:
  A) QKV projections for its 4 chunks (fp32r matmuls), bounce to DRAM.
  B) Per chunk: block-sparse attention. Scores computed transposed
     (S^T strips, k on partitions) in fp32r; exp on ScalarE (scale=1/8
     folded in) to bf16 E strips; AV matmuls in bf16 with a ones column
     appended to V giving softmax sums for free; normalization via
     reciprocal + DMA partition-broadcast.
  C) Partial output projection y^T = sum_h Wo_h O_h^T with head pairs
     stacked on partitions (K=128, bf16).

The block mask (band + global cols 0/31 + 3 random blocks) is known at
trace time from src_blocks/tgt_blocks, so the sparsity plan is specialized
per call.
"""

import numpy as np
import ml_dtypes

import concourse.bass as bass
import concourse.mybir as mybir
import concourse.tile as tile
from concourse import bacc
from concourse.bass_utils import run_bass_kernel_spmd

B, S, DIM = 2, 2048, 1024
NHEADS, HD, BLK = 16, 64, 64
NB = S // BLK          # 32 block rows/cols
NCORES = 8
HPC = NHEADS * B // NCORES  # 4 chunks (b,h) per core
P = 128

f32 = mybir.dt.float32
f32r = mybir.dt.float32r
bf16 = mybir.dt.bfloat16

LAST_EXEC_NS = None
LAST_TRACE = None


def _block_mask(src_blocks, tgt_blocks):
    i = np.arange(NB)[:, None]
    j = np.arange(NB)[None, :]
    bm = (np.abs(i - j) <= 1) | (j == 0) | (j == NB - 1)
    bm[np.asarray(src_blocks), np.asarray(tgt_blocks)] = True
    return bm


def _plan_strips(bm):
    """Cover the active blocks with k-stacked strips.

    Strip = dict(k=[kb...] (1 or 2 k-blocks stacked on partitions),
                 q0, qn (q-block run), act [len(k), qn] bool, kind).
    Active cells are claimed exactly once across strips so softmax sums
    are exact.
    """
    claimed = np.zeros((NB, NB), bool)
    strips = []
    # global columns 0 and 31, stacked, full q range
    strips.append(dict(k=[0, NB - 1], q0=0, qn=NB,
                       act=np.ones((2, NB), bool), kind="glob"))
    claimed[:, 0] = True
    claimed[:, NB - 1] = True
    # band strips: k-pair (2m-1, 2m), q-blocks [2m-2, 2m+2)
    for m in range(1, NB // 2):
        kbs = [2 * m - 1, 2 * m]
        q0, qn = 2 * m - 2, 4
        act = np.zeros((2, qn), bool)
        for ki, k in enumerate(kbs):
            for qi in range(qn):
                q = q0 + qi
                if bm[q, k] and not claimed[q, k]:
                    act[ki, qi] = True
                    claimed[q, k] = True
        strips.append(dict(k=kbs, q0=q0, qn=qn, act=act, kind="band"))
    # leftover random blocks
    rem = np.argwhere(bm & ~claimed)
    byk = {}
    for q, k in rem:
        byk.setdefault(int(k), []).append(int(q))
    for k, qs in sorted(byk.items()):
        qs = sorted(qs)
        while qs:
            q0 = min(max(qs[0] - 1, 0), NB - 4)
            qn = 4
            act = np.zeros((1, qn), bool)
            rest = []
            for q in qs:
                if q0 <= q < q0 + qn:
                    act[0, q - q0] = True
                    claimed[q, k] = True
                else:
                    rest.append(q)
            qs = rest
            strips.append(dict(k=[k], q0=q0, qn=qn, act=act, kind="extra"))
    return strips


def _build_program(strips):
    nc = bacc.Bacc("TRN2", target_bir_lowering=False, debug=False,
                   num_devices=NCORES)

    # ---- per-core external inputs ----
    d_xt = nc.dram_tensor("xt", [HPC, P, DIM], f32r, kind="ExternalInput")
    d_wq = nc.dram_tensor("wq", [P, 8 * DIM], f32r, kind="ExternalInput")
    d_wk = nc.dram_tensor("wk", [P, 8 * DIM], f32r, kind="ExternalInput")
    d_wv = nc.dram_tensor("wv", [P, 8 * DIM], f32r, kind="ExternalInput")
    d_bq = nc.dram_tensor("bq", [1, DIM], f32, kind="ExternalInput")
    d_bk = nc.dram_tensor("bk", [1, DIM], f32, kind="ExternalInput")
    d_bv = nc.dram_tensor("bv", [1, DIM], f32, kind="ExternalInput")
    d_wo = nc.dram_tensor("wo", [2, P, DIM], bf16, kind="ExternalInput")
    d_yt = nc.dram_tensor("yt", [DIM, S], f32, kind="ExternalOutput")

    with tile.TileContext(nc) as tc:
        _emit(nc, tc, strips, d_xt, (d_wq, d_wk, d_wv),
              (d_bq, d_bk, d_bv), d_wo, d_yt)
    nc.compile()
    return nc


def _emit(nc, tc, strips, d_xt, d_w, d_b, d_wo, d_yt):
    from contextlib import ExitStack
    with ExitStack() as ctx:
        psA = ctx.enter_context(tc.tile_pool(name="psA", bufs=2, space="PSUM"))
        psS = ctx.enter_context(tc.tile_pool(name="psS", bufs=2, space="PSUM"))
        psOT = ctx.enter_context(tc.tile_pool(name="psOT", bufs=1, space="PSUM"))
        dram = ctx.enter_context(tc.tile_pool(name="dram", bufs=1, space="DRAM"))
        sbB = ctx.enter_context(tc.tile_pool(name="sbB", bufs=1))
        sbW = ctx.enter_context(tc.tile_pool(name="sbW", bufs=2))
        sbN = ctx.enter_context(tc.tile_pool(name="sbN", bufs=1))

        # DRAM scratch: per-chunk projection bounces
        dlin = {}
        for nm, dt_ in (("q", f32), ("k", f32), ("v", bf16)):
            dlin[nm] = [dram.tile([P, DIM], dt_, tag=f"d{nm}{i}",
                                  name=f"d{nm}{i}")
                        for i in range(HPC)]

        # ---------------- Phase A: QKV projections (proj-major) ----------
        with tc.tile_pool(name="wp", bufs=2) as wp, \
             tc.tile_pool(name="xp", bufs=1) as xp, \
             tc.tile_pool(name="bp", bufs=2) as bp, \
             tc.tile_pool(name="lp", bufs=2) as lp:
            xtiles = [xp.tile([P, DIM], f32r, tag=f"xt{i}", name=f"xt{i}")
                      for i in range(HPC)]
            for i in range(HPC):
                nc.sync.dma_start(xtiles[i][:], d_xt[i])
            for nm, dw, db in zip("qkv", d_w, d_b):
                w = wp.tile([P, 8 * DIM], f32r, tag="w")
                nc.sync.dma_start(w[:], dw[:])
                bt = bp.tile([P, DIM], f32, tag="b")
                nc.sync.dma_start(bt[:], db[:].to_broadcast((P, DIM)))
                for i in range(HPC):
                    xt = xtiles[i]
                    lint = lp.tile([P, DIM], bf16 if nm == "v" else f32,
                                   tag="lin")
                    for nb2 in range(2):
                        ps = psA.tile([P, 512], f32, tag="mm512")
                        for kt in range(8):
                            nc.tensor.matmul(
                                ps[:],
                                lhsT=xt[:, kt * P:(kt + 1) * P],
                                rhs=w[:, kt * DIM + nb2 * 512:
                                      kt * DIM + nb2 * 512 + 512],
                                start=(kt == 0), stop=(kt == 7))
                        nc.vector.tensor_add(
                            lint[:, nb2 * 512:(nb2 + 1) * 512], ps[:],
                            bt[:, nb2 * 512:(nb2 + 1) * 512])
                    nc.sync.dma_start(dlin[nm][i][:], lint[:])

        # Wo slices for phase C (loaded early, small)
        wob = sbB.tile([P, 2 * DIM], bf16, tag="wob")
        nc.sync.dma_start(wob[:, 0:DIM], d_wo[0])
        nc.sync.dma_start(wob[:, DIM:2 * DIM], d_wo[1])

        # O2 tiles: head-pair-stacked normalized O^T, consumed by phase C
        o2 = [sbB.tile([P, S], bf16, tag=f"o2_{a}", name=f"o2_{a}")
              for a in range(2)]

        # ---------------- Phase B: attention per chunk ----------------
        for i in range(HPC):
            qt = sbW.tile([64, S], f32r, tag="qt")
            nc.sync.dma_start(
                qt[:].rearrange("d (t c) -> d t c", c=16),
                dlin["q"][i][:].bitcast(f32r).rearrange("t (c d) -> d t c", d=64))
            kt_ = sbW.tile([64, S], f32r, tag="kt")
            nc.sync.dma_start(
                kt_[:].rearrange("d (t c) -> d t c", c=16),
                dlin["k"][i][:].bitcast(f32r).rearrange("t (c d) -> d t c", d=64))
            ktg = sbN.tile([64, P], f32r, tag="ktg")
            nc.sync.dma_start(
                ktg[:, 0:64].rearrange("d (t c) -> d t c", c=16),
                dlin["k"][i][0:4].bitcast(f32r).rearrange("t (c d) -> d t c", d=64))
            nc.sync.dma_start(
                ktg[:, 64:128].rearrange("d (t c) -> d t c", c=16),
                dlin["k"][i][124:128].bitcast(f32r).rearrange("t (c d) -> d t c", d=64))
            # V in band-pair layout: group g <-> k-blocks (2g+1, 2g+2)
            v2b = sbN.tile([P, 15 * 65], bf16, tag="v2b")
            nc.sync.dma_start(
                v2b[:].rearrange("p (g e) -> p g e", e=65)[:, :, 0:64],
                dlin["v"][i][4:124].rearrange("(g a) (b d) -> (a b) g d",
                                              a=8, d=64))
            nc.vector.memset(
                v2b[:].rearrange("p (g e) -> p g e", e=65)[:, :, 64:65], 1.0)
            # V glob pair: rows 0:64 = block 0, 64:128 = block 31, + ones col
            v2g = sbN.tile([P, 65], bf16, tag="v2g")
            nc.sync.dma_start(
                v2g[0:64, 0:64],
                dlin["v"][i][0:4].rearrange("t (c d) -> (t c) d", d=64))
            nc.sync.dma_start(
                v2g[64:128, 0:64],
                dlin["v"][i][124:128].rearrange("t (c d) -> (t c) d", d=64))
            nc.vector.memset(v2g[:, 64:65], 1.0)

            # --- strips: QK -> exp -> AV (interleaved) ---
            # AV accumulates O~^T (+ sums row 64) into psum [65, S].
            # Per psum bank: glob piece first (start=True); last piece per
            # bank gets stop=True (plan computed below).
            ot = psOT.tile([65, S], f32, tag="ot")
            npieces = [0] * 4   # total AV pieces per bank
            for st in strips:
                q = st["q0"] * BLK
                qhi = (st["q0"] + st["qn"]) * BLK
                while q < qhi:
                    bk2 = q // 512
                    qe = min(qhi, (bk2 + 1) * 512)
                    npieces[bk2] += 1
                    q = qe
            done = [0] * 4

            def av_pieces(st, lhs, et, pb, rows):
                qlo = st["q0"] * BLK
                qhi = (st["q0"] + st["qn"]) * BLK
                q = qlo
                while q < qhi:
                    bk2 = q // 512
                    qe = min(qhi, (bk2 + 1) * 512)
                    nc.tensor.matmul(
                        ot[0:65, q:qe], lhsT=lhs,
                        rhs=et[pb:pb + rows, q - qlo:qe - qlo],
                        start=(done[bk2] == 0),
                        stop=(done[bk2] == npieces[bk2] - 1))
                    done[bk2] += 1
                    q = qe

            with tc.tile_pool(name=f"pe{i}", bufs=1) as pe:
                for si, st in enumerate(strips):
                    qlo, qn = st["q0"] * BLK, st["qn"] * BLK
                    if st["kind"] == "glob":
                        eg = pe.tile([P, S], bf16, tag="eg", name="eg")
                        for bk2 in range(4):
                            pss = psS.tile([P, 512], f32, tag="s")
                            nc.tensor.matmul(
                                pss[:], lhsT=ktg[:],
                                rhs=qt[:, bk2 * 512:(bk2 + 1) * 512],
                                start=True, stop=True)
                            nc.scalar.activation(
                                eg[:, bk2 * 512:(bk2 + 1) * 512], pss[:],
                                mybir.ActivationFunctionType.Exp, scale=0.125)
                        av_pieces(st, v2g[:], eg, 0, 128)
                    elif st["kind"] == "band":
                        k0 = st["k"][0] * BLK
                        em = pe.tile([P, 256], bf16, tag=f"es{si}",
                                     name=f"es{si}")
                        pss = psS.tile([P, 512], f32, tag="s")
                        nc.tensor.matmul(
                            pss[:, 0:qn], lhsT=kt_[:, k0:k0 + 128],
                            rhs=qt[:, qlo:qlo + qn],
                            start=True, stop=True)
                        nc.scalar.activation(
                            em[:, 0:qn], pss[:, 0:qn],
                            mybir.ActivationFunctionType.Exp, scale=0.125)
                        for ki in range(2):
                            for qi in range(st["qn"]):
                                if not st["act"][ki, qi]:
                                    nc.vector.memset(
                                        em[ki * 64:(ki + 1) * 64,
                                           qi * 64:(qi + 1) * 64], 0.0)
                        g = (st["k"][0] - 1) // 2
                        av_pieces(st, v2b[:, g * 65:(g + 1) * 65], em, 0, 128)
                    else:  # extra: single k-block, all at partition base 0
                        kb = st["k"][0]
                        vx = pe.tile([64, 65], bf16, tag=f"vx{si}",
                                     name=f"vx{si}")
                        nc.sync.dma_start(
                            vx[:, 0:64],
                            dlin["v"][i][kb * 4:kb * 4 + 4].rearrange(
                                "t (c d) -> (t c) d", d=64))
                        nc.vector.memset(vx[:, 64:65], 1.0)
                        ex = pe.tile([P, 256], bf16, tag=f"es{si}",
                                     name=f"es{si}")
                        nc.vector.memset(ex[0:64, 0:qn], 0.0)
                        pss = psS.tile([P, 512], f32, tag="s")
                        nc.tensor.matmul(
                            pss[0:64, 0:qn],
                            lhsT=kt_[:, kb * BLK:kb * BLK + 64],
                            rhs=qt[:, qlo:qlo + qn],
                            start=True, stop=True)
                        for qi in range(st["qn"]):
                            if st["act"][0, qi]:
                                nc.scalar.activation(
                                    ex[0:64, qi * 64:(qi + 1) * 64],
                                    pss[0:64, qi * 64:(qi + 1) * 64],
                                    mybir.ActivationFunctionType.Exp, scale=0.125)
                        av_pieces(st, vx[:], ex, 0, 64)

            # --- normalize -> O2 ---
            sums = sbN.tile([65, S], f32, tag="sums")
            nc.vector.reciprocal(sums[64:65, :], ot[64:65, :])
            drr = dram.tile([1, S], f32, tag=f"drr{i % 2}")
            nc.sync.dma_start(drr[:], sums[64:65, :])
            rbc = sbN.tile([64, S], f32, tag="rbc")
            nc.sync.dma_start(rbc[:], drr[:].to_broadcast((64, S)))
            a, half = i // 2, i % 2
            if half == 0:
                nc.vector.tensor_mul(o2[a][0:64, :], ot[0:64, :], rbc[:])
            else:
                o2t = sbN.tile([64, S], bf16, tag="o2t")
                nc.vector.tensor_mul(o2t[:], ot[0:64, :], rbc[:])
                nc.sync.dma_start(o2[a][64:128, :], o2t[:])

        # ---------------- Phase C: partial output projection ----------------
        with tc.tile_pool(name="yp", bufs=3) as yp:
            for qb in range(4):
                for mt in range(8):
                    ps = psA.tile([P, 512], f32, tag="mm512")
                    for a in range(2):
                        nc.tensor.matmul(
                            ps[:],
                            lhsT=wob[:, a * DIM + mt * P: a * DIM + (mt + 1) * P],
                            rhs=o2[a][:, qb * 512:(qb + 1) * 512],
                            start=(a == 0), stop=(a == 1))
                    yt = yp.tile([P, 512], f32, tag="yt")
                    nc.scalar.copy(yt[:], ps[:])
                    nc.sync.dma_start(
                        d_yt[mt * P:(mt + 1) * P, qb * 512:(qb + 1) * 512],
                        yt[:])


def kernel(x, Wq, bq, Wk, bk, Wv, bv, Wo, bo, src_blocks, tgt_blocks,
           _trace=False):
    global LAST_EXEC_NS, LAST_TRACE
    x = np.asarray(x, np.float32)
    bm = _block_mask(np.asarray(src_blocks), np.asarray(tgt_blocks))
    strips = _plan_strips(bm)
    nc = _build_program(strips)

    # host-side shard prep
    # W layout for rhs: w[p, kt*1024 + j] = W[j, kt*128 + p]
    def w_rhs(W):
        Wt = np.ascontiguousarray(np.asarray(W, np.float32).T)  # [in, out]
        return np.ascontiguousarray(
            Wt.reshape(8, P, DIM).transpose(1, 0, 2).reshape(P, 8 * DIM))

    wq_h, wk_h, wv_h = w_rhs(Wq), w_rhs(Wk), w_rhs(Wv)
    WoT = np.asarray(Wo, np.float32).T  # [in(=64*head), out]
    x4 = x.reshape(B, NHEADS, P, DIM)

    in_maps = []
    for c in range(NCORES):
        b = c // 4
        h0 = 4 * (c % 4)
        xc = x4[b, h0:h0 + 4]                       # [4, 128, 1024]
        xt = np.ascontiguousarray(xc.transpose(0, 2, 1))  # [4, 1024, 128]
        # xt dram layout [4, 128, 8*128]: xts[i, p, kt*128+t] = x[t, kt*128+p]
        xts = np.ascontiguousarray(
            xt.reshape(HPC, 8, P, P).transpose(0, 2, 1, 3).reshape(HPC, P, 8 * P))
        wo_c = np.zeros((2, P, DIM), ml_dtypes.bfloat16)
        for a in range(2):
            r0 = 64 * (h0 + 2 * a)
            wo_c[a] = WoT[r0:r0 + 128].astype(ml_dtypes.bfloat16)
        in_maps.append({
            "xt": xts,
            "wq": wq_h, "wk": wk_h, "wv": wv_h,
            "bq": np.asarray(bq, np.float32).reshape(1, DIM),
            "bk": np.asarray(bk, np.float32).reshape(1, DIM),
            "bv": np.asarray(bv, np.float32).reshape(1, DIM),
            "wo": wo_c,
        })

    if _trace:
        try:
            import sys
            sys.path.insert(0, "/root/problem/work")
            import ntff_shim
            ntff_shim.install()
        except Exception:
            pass
    res = run_bass_kernel_spmd(nc, in_maps, core_ids=list(range(NCORES)),
                               trace=_trace)
    LAST_EXEC_NS = res.exec_time_ns
    LAST_TRACE = (res.instructions_and_trace[1]
                  if res.instructions_and_trace else None)

    y = np.zeros((B, S, DIM), np.float32)
    for c in range(NCORES):
        y[c // 4] += res.results[c]["yt"].T
    y += np.asarray(bo, np.float32)
    return y


# revision 13
# speedup vs baseline: 3.1011x; 3.1011x over previous
"""BigBird attention (faithful .view-split variant) on 8 Trainium2 NeuronCores.

Sharding: the reference's `.reshape(B, H, S, hd)` head-split makes each
(batch, head) attend over a [2048, 64] row-major reshape of a 128-token
chunk's [128, 1024] projection. The 2*16 = 32 (b,h) pairs are sharded 4 per
core (batch x head parallel). The output projection is computed per-core as
a partial sum over its 4 heads (row-parallel over Wo), partials are summed
on the host.

Per core:
  A) QKV projections for its 4 chunks (fp32r matmuls), bounce to DRAM.
  B) Per chunk: block-sparse attention. Scores computed transposed
     (S^T strips, k on partitions) in fp32r; exp on ScalarE (scale=1/8
     folded in) to bf16 E strips; AV matmuls in bf16 with a ones column
     appended to V giving softmax sums for free; normalization via
     reciprocal + DMA partition-broadcast.
  C) Partial output projection y^T = sum_h Wo_h O_h^T with head pairs
     stacked on partitions (K=128, bf16).

The block mask (band + global cols 0/31 + 3 random blocks) is known at
trace time from src_blocks/tgt_blocks, so the sparsity plan is specialized
per call.
"""

import numpy as np
import ml_dtypes

import concourse.bass as bass
import concourse.mybir as mybir
import concourse.tile as tile
from concourse import bacc
from concourse.bass_utils import run_bass_kernel_spmd

B, S, DIM = 2, 2048, 1024
NHEADS, HD, BLK = 16, 64, 64
NB = S // BLK          # 32 block rows/cols
NCORES = 8
HPC = NHEADS * B // NCORES  # 4 chunks (b,h) per core
P = 128

f32 = mybir.dt.float32
f32r = mybir.dt.float32r
bf16 = mybir.dt.bfloat16

LAST_EXEC_NS = None
LAST_TRACE = None


def _block_mask(src_blocks, tgt_blocks):
    i = np.arange(NB)[:, None]
    j = np.arange(NB)[None, :]
    bm = (np.abs(i - j) <= 1) | (j == 0) | (j == NB - 1)
    bm[np.asarray(src_blocks), np.asarray(tgt_blocks)] = True
    return bm


def _plan_strips(bm):
    """Cover the active blocks with k-stacked strips.

    Strip = dict(k=[kb...] (1 or 2 k-blocks stacked on partitions),
                 q0, qn (q-block run), act [len(k), qn] bool, kind).
    Active cells are claimed exactly once across strips so softmax sums
    are exact.
    """
    claimed = np.zeros((NB, NB), bool)
    strips = []
    # global columns 0 and 31, stacked, full q range
    strips.append(dict(k=[0, NB - 1], q0=0, qn=NB,
                       act=np.ones((2, NB), bool), kind="glob"))
    claimed[:, 0] = True
    claimed[:, NB - 1] = True
    # band strips: k-pair (2m-1, 2m), q-blocks [2m-2, 2m+2)
    for m in range(1, NB // 2):
        kbs = [2 * m - 1, 2 * m]
        q0, qn = 2 * m - 2, 4
        act = np.zeros((2, qn), bool)
        for ki, k in enumerate(kbs):
            for qi in range(qn):
                q = q0 + qi
                if bm[q, k] and not claimed[q, k]:
                    act[ki, qi] = True
                    claimed[q, k] = True
        strips.append(dict(k=kbs, q0=q0, qn=qn, act=act, kind="band"))
    # leftover random blocks
    rem = np.argwhere(bm & ~claimed)
    byk = {}
    for q, k in rem:
        byk.setdefault(int(k), []).append(int(q))
    for k, qs in sorted(byk.items()):
        qs = sorted(qs)
        while qs:
            q0 = min(max(qs[0] - 1, 0), NB - 4)
            qn = 4
            act = np.zeros((1, qn), bool)
            rest = []
            for q in qs:
                if q0 <= q < q0 + qn:
                    act[0, q - q0] = True
                    claimed[q, k] = True
                else:
                    rest.append(q)
            qs = rest
            strips.append(dict(k=[k], q0=q0, qn=qn, act=act, kind="extra"))
    return strips


def _build_program(strips):
    nc = bacc.Bacc("TRN2", target_bir_lowering=False, debug=False,
                   num_devices=NCORES)

    # ---- per-core external inputs ----
    d_xt = nc.dram_tensor("xt", [HPC, P, DIM], f32r, kind="ExternalInput")
    d_wq = nc.dram_tensor("wq", [P, 8 * DIM], f32r, kind="ExternalInput")
    d_wk = nc.dram_tensor("wk", [P, 8 * DIM], f32r, kind="ExternalInput")
    d_wv = nc.dram_tensor("wv", [P, 8 * DIM], f32r, kind="ExternalInput")
    d_bq = nc.dram_tensor("bq", [1, DIM], f32, kind="ExternalInput")
    d_bk = nc.dram_tensor("bk", [1, DIM], f32, kind="ExternalInput")
    d_bv = nc.dram_tensor("bv", [1, DIM], f32, kind="ExternalInput")
    d_wo = nc.dram_tensor("wo", [2, P, DIM], bf16, kind="ExternalInput")
    d_yt = nc.dram_tensor("yt", [DIM, S], f32, kind="ExternalOutput")

    with tile.TileContext(nc) as tc:
        _emit(nc, tc, strips, d_xt, (d_wq, d_wk, d_wv),
              (d_bq, d_bk, d_bv), d_wo, d_yt)
    nc.compile()
    return nc


def _emit(nc, tc, strips, d_xt, d_w, d_b, d_wo, d_yt):
    from contextlib import ExitStack
    with ExitStack() as ctx:
        psA = ctx.enter_context(tc.tile_pool(name="psA", bufs=2, space="PSUM"))
        psS = ctx.enter_context(tc.tile_pool(name="psS", bufs=2, space="PSUM"))
        psOT = ctx.enter_context(tc.tile_pool(name="psOT", bufs=1, space="PSUM"))
        dram = ctx.enter_context(tc.tile_pool(name="dram", bufs=1, space="DRAM"))
        sbB = ctx.enter_context(tc.tile_pool(name="sbB", bufs=1))
        sbW = ctx.enter_context(tc.tile_pool(name="sbW", bufs=2))
        sbN = ctx.enter_context(tc.tile_pool(name="sbN", bufs=1))

        # DRAM scratch: per-chunk projection bounces
        dlin = {}
        for nm, shp in (("q", [S, P]), ("k", [S, P]), ("v", [P, DIM])):
            dlin[nm] = [dram.tile(shp, bf16, tag=f"d{nm}{i}",
                                  name=f"d{nm}{i}")
                        for i in range(HPC)]

        # ---------------- Phase A: QKV projections (proj-major) ----------
        with tc.tile_pool(name="wp", bufs=2) as wp, \
             tc.tile_pool(name="xp", bufs=1) as xp, \
             tc.tile_pool(name="bp", bufs=2) as bp, \
             tc.tile_pool(name="lp", bufs=2) as lp:
            xtiles = [xp.tile([P, DIM], f32r, tag=f"xt{i}", name=f"xt{i}")
                      for i in range(HPC)]
            for i in range(HPC):
                nc.sync.dma_start(xtiles[i][:], d_xt[i])
            for nm, dw, db in zip("qkv", d_w, d_b):
                w = wp.tile([P, 8 * DIM], f32r, tag="w")
                nc.sync.dma_start(w[:], dw[:])
                bt = bp.tile([P, DIM], f32, tag="b")
                nc.sync.dma_start(bt[:], db[:].to_broadcast((P, DIM)))
                if nm == "v":
                    lint = lp.tile([P, DIM], bf16, tag="linv", name="lintv")
                else:
                    # d-axis padded to 128 (zeros) so the bounce is DMA-
                    # transposable: dram layout [s', 128] = [t, (c, d|pad)]
                    lint = lp.tile([P, 2 * DIM], bf16, tag=f"lin{nm}",
                                   name=f"lint{nm}")
                    nc.vector.memset(
                        lint[:].rearrange("p (c x) -> p c x",
                                          x=P)[:, :, 64:P], 0.0)
                for i in range(HPC):
                    xt = xtiles[i]
                    for nb2 in range(2):
                        ps = psA.tile([P, 512], f32, tag="mm512")
                        for kt in range(8):
                            nc.tensor.matmul(
                                ps[:],
                                lhsT=xt[:, kt * P:(kt + 1) * P],
                                rhs=w[:, kt * DIM + nb2 * 512:
                                      kt * DIM + nb2 * 512 + 512],
                                start=(kt == 0), stop=(kt == 7))
                        if nm == "v":
                            out_ap = lint[:, nb2 * 512:(nb2 + 1) * 512
                                          ].rearrange("p (c d) -> p c d", d=64)
                        else:
                            out_ap = lint[:].rearrange(
                                "p (c x) -> p c x",
                                x=P)[:, nb2 * 8:(nb2 + 1) * 8, 0:64]
                        nc.vector.tensor_add(
                            out_ap,
                            ps[:].rearrange("p (c d) -> p c d", d=64),
                            bt[:, nb2 * 512:(nb2 + 1) * 512].rearrange(
                                "p (c d) -> p c d", d=64))
                    nc.sync.dma_start(dlin[nm][i][:], lint[:])

        # Wo slices for phase C (loaded early, small)
        wob = sbB.tile([P, 2 * DIM], bf16, tag="wob")
        nc.sync.dma_start(wob[:, 0:DIM], d_wo[0])
        nc.sync.dma_start(wob[:, DIM:2 * DIM], d_wo[1])

        # O2 tiles: head-pair-stacked normalized O^T, consumed by phase C
        o2 = [sbB.tile([P, S], bf16, tag=f"o2_{a}", name=f"o2_{a}")
              for a in range(2)]

        # ---------------- Phase B: attention per chunk ----------------
        for i in range(HPC):
            qt = sbW.tile([P, S], bf16, tag="qt")
            nc.sync.dma_start(qt[:], dlin["q"][i][:], transpose=True)
            kt_ = sbW.tile([P, S], bf16, tag="kt")
            nc.sync.dma_start(kt_[:], dlin["k"][i][:], transpose=True)
            ktg = sbN.tile([P, P], bf16, tag="ktg")
            nc.sync.dma_start(ktg[:, 0:64], dlin["k"][i][0:64], transpose=True)
            nc.sync.dma_start(ktg[:, 64:128], dlin["k"][i][S - 64:S],
                              transpose=True)
            # V in band-pair layout: group g <-> k-blocks (2g+1, 2g+2)
            v2b = sbN.tile([P, 15 * 65], bf16, tag="v2b")
            nc.sync.dma_start(
                v2b[:].rearrange("p (g e) -> p g e", e=65)[:, :, 0:64],
                dlin["v"][i][4:124].rearrange("(g a) (b d) -> (a b) g d",
                                              a=8, d=64))
            nc.vector.memset(
                v2b[:].rearrange("p (g e) -> p g e", e=65)[:, :, 64:65], 1.0)
            # V glob pair: rows 0:64 = block 0, 64:128 = block 31, + ones col
            v2g = sbN.tile([P, 65], bf16, tag="v2g")
            nc.sync.dma_start(
                v2g[0:64, 0:64],
                dlin["v"][i][0:4].rearrange("t (c d) -> (t c) d", d=64))
            nc.sync.dma_start(
                v2g[64:128, 0:64],
                dlin["v"][i][124:128].rearrange("t (c d) -> (t c) d", d=64))
            nc.vector.memset(v2g[:, 64:65], 1.0)

            # --- strips: QK -> exp -> AV (interleaved) ---
            # AV accumulates O~^T (+ sums row 64) into psum [65, S].
            # Per psum bank: glob piece first (start=True); last piece per
            # bank gets stop=True (plan computed below).
            ot = psOT.tile([65, S], f32, tag="ot")
            npieces = [0] * 4   # total AV pieces per bank
            for st in strips:
                q = st["q0"] * BLK
                qhi = (st["q0"] + st["qn"]) * BLK
                while q < qhi:
                    bk2 = q // 512
                    qe = min(qhi, (bk2 + 1) * 512)
                    npieces[bk2] += 1
                    q = qe
            done = [0] * 4

            def av_pieces(st, lhs, et, pb, rows):
                qlo = st["q0"] * BLK
                qhi = (st["q0"] + st["qn"]) * BLK
                q = qlo
                while q < qhi:
                    bk2 = q // 512
                    qe = min(qhi, (bk2 + 1) * 512)
                    nc.tensor.matmul(
                        ot[0:65, q:qe], lhsT=lhs,
                        rhs=et[pb:pb + rows, q - qlo:qe - qlo],
                        start=(done[bk2] == 0),
                        stop=(done[bk2] == npieces[bk2] - 1))
                    done[bk2] += 1
                    q = qe

            with tc.tile_pool(name=f"pe{i}", bufs=1) as pe:
                for si, st in enumerate(strips):
                    qlo, qn = st["q0"] * BLK, st["qn"] * BLK
                    if st["kind"] == "glob":
                        eg = pe.tile([P, S], bf16, tag="eg", name="eg")
                        for bk2 in range(4):
                            pss = psS.tile([P, 512], f32, tag="s")
                            nc.tensor.matmul(
                                pss[:], lhsT=ktg[0:64, :],
                                rhs=qt[0:64, bk2 * 512:(bk2 + 1) * 512],
                                start=True, stop=True)
                            nc.scalar.activation(
                                eg[:, bk2 * 512:(bk2 + 1) * 512], pss[:],
                                mybir.ActivationFunctionType.Exp, scale=0.125)
                        av_pieces(st, v2g[:], eg, 0, 128)
                    elif st["kind"] == "band":
                        k0 = st["k"][0] * BLK
                        em = pe.tile([P, 256], bf16, tag=f"es{si}",
                                     name=f"es{si}")
                        pss = psS.tile([P, 512], f32, tag="s")
                        nc.tensor.matmul(
                            pss[:, 0:qn], lhsT=kt_[0:64, k0:k0 + 128],
                            rhs=qt[0:64, qlo:qlo + qn],
                            start=True, stop=True)
                        nc.scalar.activation(
                            em[:, 0:qn], pss[:, 0:qn],
                            mybir.ActivationFunctionType.Exp, scale=0.125)
                        for ki in range(2):
                            for qi in range(st["qn"]):
                                if not st["act"][ki, qi]:
                                    nc.vector.memset(
                                        em[ki * 64:(ki + 1) * 64,
                                           qi * 64:(qi + 1) * 64], 0.0)
                        g = (st["k"][0] - 1) // 2
                        av_pieces(st, v2b[:, g * 65:(g + 1) * 65], em, 0, 128)
                    else:  # extra: single k-block, all at partition base 0
                        kb = st["k"][0]
                        vx = pe.tile([64, 65], bf16, tag=f"vx{si}",
                                     name=f"vx{si}")
                        nc.sync.dma_start(
                            vx[:, 0:64],
                            dlin["v"][i][kb * 4:kb * 4 + 4].rearrange(
                                "t (c d) -> (t c) d", d=64))
                        nc.vector.memset(vx[:, 64:65], 1.0)
                        ex = pe.tile([P, 256], bf16, tag=f"es{si}",
                                     name=f"es{si}")
                        nc.vector.memset(ex[0:64, 0:qn], 0.0)
                        pss = psS.tile([P, 512], f32, tag="s")
                        nc.tensor.matmul(
                            pss[0:64, 0:qn],
                            lhsT=kt_[0:64, kb * BLK:kb * BLK + 64],
                            rhs=qt[0:64, qlo:qlo + qn],
                            start=True, stop=True)
                        for qi in range(st["qn"]):
                            if st["act"][0, qi]:
                                nc.scalar.activation(
                                    ex[0:64, qi * 64:(qi + 1) * 64],
                                    pss[0:64, qi * 64:(qi + 1) * 64],
                                    mybir.ActivationFunctionType.Exp, scale=0.125)
                        av_pieces(st, vx[:], ex, 0, 64)

            # --- normalize -> O2 ---
            # reshape the psum sums row to [128, 16] so reciprocal runs on
            # all lanes, then broadcast 1/s across 64 partitions via DRAM
            srow = sbN.tile([65, S], f32, tag="srow")
            nc.scalar.copy(srow[64:65, :], ot[64:65, :])
            dsum = dram.tile([1, S], f32, tag=f"dsum{i % 2}",
                             name=f"dsum{i % 2}")
            nc.sync.dma_start(dsum[:], srow[64:65, :])
            ssum = sbN.tile([P, 16], f32, tag="ssum")
            nc.sync.dma_start(ssum[:],
                              dsum[:].rearrange("o (p f) -> (o p) f", f=16))
            rr = sbN.tile([P, 16], f32, tag="rr")
            nc.vector.reciprocal(rr[:], ssum[:])
            drr = dram.tile([1, S], f32, tag=f"drr{i % 2}",
                            name=f"drr{i % 2}")
            nc.sync.dma_start(drr[:].rearrange("o (p f) -> (o p) f", f=16),
                              rr[:])
            rbc = sbN.tile([64, S], f32, tag="rbc")
            nc.sync.dma_start(rbc[:], drr[:].to_broadcast((64, S)))
            a, half = i // 2, i % 2
            if half == 0:
                nc.vector.tensor_mul(o2[a][0:64, :], ot[0:64, :], rbc[:])
            else:
                o2t = sbN.tile([64, S], bf16, tag="o2t")
                nc.vector.tensor_mul(o2t[:], ot[0:64, :], rbc[:])
                nc.sync.dma_start(o2[a][64:128, :], o2t[:])

        # ---------------- Phase C: partial output projection ----------------
        with tc.tile_pool(name="yp", bufs=3) as yp:
            for qb in range(4):
                for mt in range(8):
                    ps = psA.tile([P, 512], f32, tag="mm512")
                    for a in range(2):
                        nc.tensor.matmul(
                            ps[:],
                            lhsT=wob[:, a * DIM + mt * P: a * DIM + (mt + 1) * P],
                            rhs=o2[a][:, qb * 512:(qb + 1) * 512],
                            start=(a == 0), stop=(a == 1))
                    yt = yp.tile([P, 512], f32, tag="yt")
                    nc.scalar.copy(yt[:], ps[:])
                    nc.sync.dma_start(
                        d_yt[mt * P:(mt + 1) * P, qb * 512:(qb + 1) * 512],
                        yt[:])


def kernel(x, Wq, bq, Wk, bk, Wv, bv, Wo, bo, src_blocks, tgt_blocks,
           _trace=False):
    global LAST_EXEC_NS, LAST_TRACE
    x = np.asarray(x, np.float32)
    bm = _block_mask(np.asarray(src_blocks), np.asarray(tgt_blocks))
    strips = _plan_strips(bm)
    nc = _build_program(strips)

    # host-side shard prep
    # W layout for rhs: w[p, kt*1024 + j] = W[j, kt*128 + p]
    def w_rhs(W):
        Wt = np.ascontiguousarray(np.asarray(W, np.float32).T)  # [in, out]
        return np.ascontiguousarray(
            Wt.reshape(8, P, DIM).transpose(1, 0, 2).reshape(P, 8 * DIM))

    wq_h, wk_h, wv_h = w_rhs(Wq), w_rhs(Wk), w_rhs(Wv)
    WoT = np.asarray(Wo, np.float32).T  # [in(=64*head), out]
    x4 = x.reshape(B, NHEADS, P, DIM)

    in_maps = []
    for c in range(NCORES):
        b = c // 4
        h0 = 4 * (c % 4)
        xc = x4[b, h0:h0 + 4]                       # [4, 128, 1024]
        xt = np.ascontiguousarray(xc.transpose(0, 2, 1))  # [4, 1024, 128]
        # xt dram layout [4, 128, 8*128]: xts[i, p, kt*128+t] = x[t, kt*128+p]
        xts = np.ascontiguousarray(
            xt.reshape(HPC, 8, P, P).transpose(0, 2, 1, 3).reshape(HPC, P, 8 * P))
        wo_c = np.zeros((2, P, DIM), ml_dtypes.bfloat16)
        for a in range(2):
            r0 = 64 * (h0 + 2 * a)
            wo_c[a] = WoT[r0:r0 + 128].astype(ml_dtypes.bfloat16)
        in_maps.append({
            "xt": xts,
            "wq": wq_h, "wk": wk_h, "wv": wv_h,
            "bq": np.asarray(bq, np.float32).reshape(1, DIM),
            "bk": np.asarray(bk, np.float32).reshape(1, DIM),
            "bv": np.asarray(bv, np.float32).reshape(1, DIM),
            "wo": wo_c,
        })

    if _trace:
        try:
            import sys
            sys.path.insert(0, "/root/problem/work")
            import ntff_shim
            ntff_shim.install()
        except Exception:
            pass
    res = run_bass_kernel_spmd(nc, in_maps, core_ids=list(range(NCORES)),
                               trace=_trace)
    LAST_EXEC_NS = res.exec_time_ns
    LAST_TRACE = (res.instructions_and_trace[1]
                  if res.instructions_and_trace else None)

    y = np.zeros((B, S, DIM), np.float32)
    for c in range(NCORES):
        y[c // 4] += res.results[c]["yt"].T
    y += np.asarray(bo, np.float32)
    return y


# revision 15
# speedup vs baseline: 3.1329x; 1.0102x over previous
"""BigBird attention (faithful .view-split variant) on 8 Trainium2 NeuronCores.

Sharding: the reference's `.reshape(B, H, S, hd)` head-split makes each
(batch, head) attend over a [2048, 64] row-major reshape of a 128-token
chunk's [128, 1024] projection. The 2*16 = 32 (b,h) pairs are sharded 4 per
core (batch x head parallel). The output projection is computed per-core as
a partial sum over its 4 heads (row-parallel over Wo), partials are summed
on the host.

Per core:
  A) QKV projections for its 4 chunks (fp32r matmuls), bounce to DRAM.
  B) Per chunk: block-sparse attention. Scores computed transposed
     (S^T strips, k on partitions) in fp32r; exp on ScalarE (scale=1/8
     folded in) to bf16 E strips; AV matmuls in bf16 with a ones column
     appended to V giving softmax sums for free; normalization via
     reciprocal + DMA partition-broadcast.
  C) Partial output projection y^T = sum_h Wo_h O_h^T with head pairs
     stacked on partitions (K=128, bf16).

The block mask (band + global cols 0/31 + 3 random blocks) is known at
trace time from src_blocks/tgt_blocks, so the sparsity plan is specialized
per call.
"""

import numpy as np
import ml_dtypes

import concourse.bass as bass
import concourse.mybir as mybir
import concourse.tile as tile
from concourse import bacc
from concourse.bass_utils import run_bass_kernel_spmd

B, S, DIM = 2, 2048, 1024
NHEADS, HD, BLK = 16, 64, 64
NB = S // BLK          # 32 block rows/cols
NCORES = 8
HPC = NHEADS * B // NCORES  # 4 chunks (b,h) per core
P = 128

f32 = mybir.dt.float32
f32r = mybir.dt.float32r
bf16 = mybir.dt.bfloat16

LAST_EXEC_NS = None
LAST_TRACE = None


def _block_mask(src_blocks, tgt_blocks):
    i = np.arange(NB)[:, None]
    j = np.arange(NB)[None, :]
    bm = (np.abs(i - j) <= 1) | (j == 0) | (j == NB - 1)
    bm[np.asarray(src_blocks), np.asarray(tgt_blocks)] = True
    return bm


def _plan_strips(bm):
    """Cover the active blocks with k-stacked strips.

    Strip = dict(k=[kb...] (1 or 2 k-blocks stacked on partitions),
                 q0, qn (q-block run), act [len(k), qn] bool, kind).
    Active cells are claimed exactly once across strips so softmax sums
    are exact.
    """
    claimed = np.zeros((NB, NB), bool)
    strips = []
    # global columns 0 and 31, stacked, full q range
    strips.append(dict(k=[0, NB - 1], q0=0, qn=NB,
                       act=np.ones((2, NB), bool), kind="glob"))
    claimed[:, 0] = True
    claimed[:, NB - 1] = True
    # band strips: k-pair (2m-1, 2m), q-blocks [2m-2, 2m+2)
    for m in range(1, NB // 2):
        kbs = [2 * m - 1, 2 * m]
        q0, qn = 2 * m - 2, 4
        act = np.zeros((2, qn), bool)
        for ki, k in enumerate(kbs):
            for qi in range(qn):
                q = q0 + qi
                if bm[q, k] and not claimed[q, k]:
                    act[ki, qi] = True
                    claimed[q, k] = True
        strips.append(dict(k=kbs, q0=q0, qn=qn, act=act, kind="band"))
    # leftover random blocks
    rem = np.argwhere(bm & ~claimed)
    byk = {}
    for q, k in rem:
        byk.setdefault(int(k), []).append(int(q))
    for k, qs in sorted(byk.items()):
        qs = sorted(qs)
        while qs:
            q0 = min(max(qs[0] - 1, 0), NB - 4)
            qn = 4
            act = np.zeros((1, qn), bool)
            rest = []
            for q in qs:
                if q0 <= q < q0 + qn:
                    act[0, q - q0] = True
                    claimed[q, k] = True
                else:
                    rest.append(q)
            qs = rest
            strips.append(dict(k=[k], q0=q0, qn=qn, act=act, kind="extra"))
    return strips


def _build_program(strips, use_bias=True):
    nc = bacc.Bacc("TRN2", target_bir_lowering=False, debug=False,
                   num_devices=NCORES)

    # ---- per-core external inputs ----
    d_xt = nc.dram_tensor("xt", [HPC, P, DIM], f32r, kind="ExternalInput")
    d_wq = nc.dram_tensor("wq", [P, 8 * DIM], f32r, kind="ExternalInput")
    d_wk = nc.dram_tensor("wk", [P, 8 * DIM], f32r, kind="ExternalInput")
    d_wv = nc.dram_tensor("wv", [P, 8 * DIM], f32r, kind="ExternalInput")
    d_bq = nc.dram_tensor("bq", [1, DIM], f32, kind="ExternalInput")
    d_bk = nc.dram_tensor("bk", [1, DIM], f32, kind="ExternalInput")
    d_bv = nc.dram_tensor("bv", [1, DIM], f32, kind="ExternalInput")
    d_wo = nc.dram_tensor("wo", [2, P, DIM], bf16, kind="ExternalInput")
    d_yt = nc.dram_tensor("yt", [DIM, S], f32, kind="ExternalOutput")

    with tile.TileContext(nc) as tc:
        _emit(nc, tc, strips, d_xt, (d_wq, d_wk, d_wv),
              (d_bq, d_bk, d_bv), d_wo, d_yt, use_bias)
    nc.compile()
    return nc


def _emit(nc, tc, strips, d_xt, d_w, d_b, d_wo, d_yt, use_bias):
    from contextlib import ExitStack
    with ExitStack() as ctx:
        psA = ctx.enter_context(tc.tile_pool(name="psA", bufs=2, space="PSUM"))
        psS = ctx.enter_context(tc.tile_pool(name="psS", bufs=2, space="PSUM"))
        psOT = ctx.enter_context(tc.tile_pool(name="psOT", bufs=2, space="PSUM"))
        dram = ctx.enter_context(tc.tile_pool(name="dram", bufs=1, space="DRAM"))
        sbB = ctx.enter_context(tc.tile_pool(name="sbB", bufs=1))
        sbW = ctx.enter_context(tc.tile_pool(name="sbW", bufs=2))
        sbN = ctx.enter_context(tc.tile_pool(name="sbN", bufs=1))

        # DRAM scratch: per-chunk projection bounces
        dlin = {}
        for nm, shp in (("q", [S, P]), ("k", [S, P]), ("v", [P, DIM])):
            dlin[nm] = [dram.tile(shp, bf16, tag=f"d{nm}{i}",
                                  name=f"d{nm}{i}")
                        for i in range(HPC)]

        # ---------------- Phase A: QKV projections (proj-major) ----------
        with tc.tile_pool(name="wp", bufs=2) as wp, \
             tc.tile_pool(name="xp", bufs=1) as xp, \
             tc.tile_pool(name="bp", bufs=2) as bp, \
             tc.tile_pool(name="lp", bufs=2) as lp:
            xtiles = [xp.tile([P, DIM], f32r, tag=f"xt{i}", name=f"xt{i}")
                      for i in range(HPC)]
            for i in range(HPC):
                nc.sync.dma_start(xtiles[i][:], d_xt[i])
            for nm, dw, db in zip("qkv", d_w, d_b):
                w = wp.tile([P, 8 * DIM], f32r, tag="w")
                for kt in range(8):
                    nc.sync.dma_start(w[:, kt * DIM:(kt + 1) * DIM],
                                      dw[:, kt * DIM:(kt + 1) * DIM])
                bt = bp.tile([P, DIM], f32, tag="b")
                nc.sync.dma_start(bt[:], db[:].to_broadcast((P, DIM)))
                if nm == "v":
                    lint = lp.tile([P, DIM], bf16, tag="linv", name="lintv")
                else:
                    # d-axis padded to 128 (zeros) so the bounce is DMA-
                    # transposable: dram layout [s', 128] = [t, (c, d|pad)]
                    lint = lp.tile([P, 2 * DIM], bf16, tag=f"lin{nm}",
                                   name=f"lint{nm}")
                    nc.vector.memset(
                        lint[:].rearrange("p (c x) -> p c x",
                                          x=P)[:, :, 64:P], 0.0)
                for i in range(HPC):
                    xt = xtiles[i]
                    for nb2 in range(2):
                        ps = psA.tile([P, 512], f32, tag="mm512")
                        for kt in range(8):
                            nc.tensor.matmul(
                                ps[:],
                                lhsT=xt[:, kt * P:(kt + 1) * P],
                                rhs=w[:, kt * DIM + nb2 * 512:
                                      kt * DIM + nb2 * 512 + 512],
                                start=(kt == 0), stop=(kt == 7))
                        if nm == "v":
                            out_ap = lint[:, nb2 * 512:(nb2 + 1) * 512
                                          ].rearrange("p (c d) -> p c d", d=64)
                        else:
                            out_ap = lint[:].rearrange(
                                "p (c x) -> p c x",
                                x=P)[:, nb2 * 8:(nb2 + 1) * 8, 0:64]
                        if use_bias:
                            nc.vector.tensor_add(
                                out_ap,
                                ps[:].rearrange("p (c d) -> p c d", d=64),
                                bt[:, nb2 * 512:(nb2 + 1) * 512].rearrange(
                                    "p (c d) -> p c d", d=64))
                        else:
                            nc.scalar.copy(
                                out_ap,
                                ps[:].rearrange("p (c d) -> p c d", d=64))
                    nc.sync.dma_start(dlin[nm][i][:], lint[:])

        # Wo slices for phase C (loaded early, small)
        wob = sbB.tile([P, 2 * DIM], bf16, tag="wob")
        nc.sync.dma_start(wob[:, 0:DIM], d_wo[0])
        nc.sync.dma_start(wob[:, DIM:2 * DIM], d_wo[1])

        # O2 tiles: head-pair-stacked normalized O^T, consumed by phase C
        o2 = [sbB.tile([P, S], bf16, tag=f"o2_{a}", name=f"o2_{a}")
              for a in range(2)]

        # ---------------- Phase B: attention per chunk ----------------
        for i in range(HPC):
            qt = sbW.tile([P, S], bf16, tag="qt")
            nc.sync.dma_start(qt[:], dlin["q"][i][:], transpose=True)
            kt_ = sbW.tile([P, S], bf16, tag="kt")
            nc.sync.dma_start(kt_[:], dlin["k"][i][:], transpose=True)
            ktg = sbN.tile([P, P], bf16, tag="ktg")
            nc.sync.dma_start(ktg[:, 0:64], dlin["k"][i][0:64], transpose=True)
            nc.sync.dma_start(ktg[:, 64:128], dlin["k"][i][S - 64:S],
                              transpose=True)
            # V in band-pair layout: group g <-> k-blocks (2g+1, 2g+2)
            v2b = sbN.tile([P, 15 * 65], bf16, tag="v2b")
            nc.sync.dma_start(
                v2b[:].rearrange("p (g e) -> p g e", e=65)[:, :, 0:64],
                dlin["v"][i][4:124].rearrange("(g a) (b d) -> (a b) g d",
                                              a=8, d=64))
            nc.vector.memset(
                v2b[:].rearrange("p (g e) -> p g e", e=65)[:, :, 64:65], 1.0)
            # V glob pair: rows 0:64 = block 0, 64:128 = block 31, + ones col
            v2g = sbN.tile([P, 65], bf16, tag="v2g")
            nc.sync.dma_start(
                v2g[0:64, 0:64],
                dlin["v"][i][0:4].rearrange("t (c d) -> (t c) d", d=64))
            nc.sync.dma_start(
                v2g[64:128, 0:64],
                dlin["v"][i][124:128].rearrange("t (c d) -> (t c) d", d=64))
            nc.vector.memset(v2g[:, 64:65], 1.0)

            # --- strips: QK -> exp -> AV (interleaved) ---
            # AV accumulates O~^T (+ sums row 64) into psum [65, S].
            # Per psum bank: glob piece first (start=True); last piece per
            # bank gets stop=True (plan computed below).
            ot_h = [psOT.tile([65, S // 2], f32, tag="ot",
                                     name=f"ot{i}_{h}") for h in range(2)]
            npieces = [0] * 4   # total AV pieces per bank
            for st in strips:
                q = st["q0"] * BLK
                qhi = (st["q0"] + st["qn"]) * BLK
                while q < qhi:
                    bk2 = q // 512
                    qe = min(qhi, (bk2 + 1) * 512)
                    npieces[bk2] += 1
                    q = qe
            done = [0] * 4

            def av_pieces(st, lhs, et, pb, rows):
                qlo = st["q0"] * BLK
                qhi = (st["q0"] + st["qn"]) * BLK
                q = qlo
                while q < qhi:
                    bk2 = q // 512
                    qe = min(qhi, (bk2 + 1) * 512)
                    nc.tensor.matmul(
                        ot_h[bk2 // 2][0:65, q - (bk2 // 2) * (S // 2):
                                       qe - (bk2 // 2) * (S // 2)],
                        lhsT=lhs,
                        rhs=et[pb:pb + rows, q - qlo:qe - qlo],
                        start=(done[bk2] == 0),
                        stop=(done[bk2] == npieces[bk2] - 1))
                    done[bk2] += 1
                    q = qe

            with tc.tile_pool(name=f"pe{i}", bufs=1) as pe:
                for si, st in enumerate(strips):
                    qlo, qn = st["q0"] * BLK, st["qn"] * BLK
                    if st["kind"] == "glob":
                        eg = pe.tile([P, S], bf16, tag="eg", name="eg")
                        for bk2 in range(4):
                            pss = psS.tile([P, 512], f32, tag="s")
                            nc.tensor.matmul(
                                pss[:], lhsT=ktg[0:64, :],
                                rhs=qt[0:64, bk2 * 512:(bk2 + 1) * 512],
                                start=True, stop=True)
                            nc.scalar.activation(
                                eg[:, bk2 * 512:(bk2 + 1) * 512], pss[:],
                                mybir.ActivationFunctionType.Exp, scale=0.125)
                        av_pieces(st, v2g[:], eg, 0, 128)
                    elif st["kind"] == "band":
                        k0 = st["k"][0] * BLK
                        em = pe.tile([P, 256], bf16, tag=f"es{si}",
                                     name=f"es{si}")
                        pss = psS.tile([P, 512], f32, tag="s")
                        nc.tensor.matmul(
                            pss[:, 0:qn], lhsT=kt_[0:64, k0:k0 + 128],
                            rhs=qt[0:64, qlo:qlo + qn],
                            start=True, stop=True)
                        nc.scalar.activation(
                            em[:, 0:qn], pss[:, 0:qn],
                            mybir.ActivationFunctionType.Exp, scale=0.125)
                        for ki in range(2):
                            for qi in range(st["qn"]):
                                if not st["act"][ki, qi]:
                                    nc.vector.memset(
                                        em[ki * 64:(ki + 1) * 64,
                                           qi * 64:(qi + 1) * 64], 0.0)
                        g = (st["k"][0] - 1) // 2
                        av_pieces(st, v2b[:, g * 65:(g + 1) * 65], em, 0, 128)
                    else:  # extra: single k-block, all at partition base 0
                        kb = st["k"][0]
                        vx = pe.tile([64, 65], bf16, tag=f"vx{si}",
                                     name=f"vx{si}")
                        nc.sync.dma_start(
                            vx[:, 0:64],
                            dlin["v"][i][kb * 4:kb * 4 + 4].rearrange(
                                "t (c d) -> (t c) d", d=64))
                        nc.vector.memset(vx[:, 64:65], 1.0)
                        ex = pe.tile([P, 256], bf16, tag=f"es{si}",
                                     name=f"es{si}")
                        nc.vector.memset(ex[0:64, 0:qn], 0.0)
                        pss = psS.tile([P, 512], f32, tag="s")
                        nc.tensor.matmul(
                            pss[0:64, 0:qn],
                            lhsT=kt_[0:64, kb * BLK:kb * BLK + 64],
                            rhs=qt[0:64, qlo:qlo + qn],
                            start=True, stop=True)
                        for qi in range(st["qn"]):
                            if st["act"][0, qi]:
                                nc.scalar.activation(
                                    ex[0:64, qi * 64:(qi + 1) * 64],
                                    pss[0:64, qi * 64:(qi + 1) * 64],
                                    mybir.ActivationFunctionType.Exp, scale=0.125)
                        av_pieces(st, vx[:], ex, 0, 64)

            # --- normalize -> O2 (per half, pipelined) ---
            # reshape the psum sums row to [128, 8] so reciprocal runs on
            # all lanes, then broadcast 1/s across 64 partitions via DRAM
            HF = S // 2
            a, half = i // 2, i % 2
            for h in range(2):
                oth = ot_h[h]
                srow = sbN.tile([65, HF], f32, tag=f"srow{h}",
                                name=f"srow{h}")
                nc.scalar.copy(srow[64:65, :], oth[64:65, :])
                dsum = dram.tile([1, HF], f32, tag=f"dsum{i % 2}{h}",
                                 name=f"dsum{i % 2}{h}")
                nc.sync.dma_start(dsum[:], srow[64:65, :])
                ssum = sbN.tile([P, 8], f32, tag=f"ssum{h}", name=f"ssum{h}")
                nc.sync.dma_start(
                    ssum[:], dsum[:].rearrange("o (p f) -> (o p) f", f=8))
                rr = sbN.tile([P, 8], f32, tag=f"rr{h}", name=f"rr{h}")
                nc.vector.reciprocal(rr[:], ssum[:])
                drr = dram.tile([1, HF], f32, tag=f"drr{i % 2}{h}",
                                name=f"drr{i % 2}{h}")
                nc.sync.dma_start(
                    drr[:].rearrange("o (p f) -> (o p) f", f=8), rr[:])
                rbc = sbN.tile([64, HF], f32, tag=f"rbc{h}", name=f"rbc{h}")
                nc.sync.dma_start(rbc[:], drr[:].to_broadcast((64, HF)))
                if half == 0:
                    nc.vector.tensor_mul(o2[a][0:64, h * HF:(h + 1) * HF],
                                         oth[0:64, :], rbc[:])
                else:
                    o2t = sbN.tile([64, HF], bf16, tag=f"o2t{h}",
                                   name=f"o2t{h}")
                    nc.vector.tensor_mul(o2t[:], oth[0:64, :], rbc[:])
                    nc.sync.dma_start(o2[a][64:128, h * HF:(h + 1) * HF],
                                      o2t[:])

        # ---------------- Phase C: partial output projection ----------------
        with tc.tile_pool(name="yp", bufs=3) as yp:
            for qb in range(4):
                for mt in range(8):
                    ps = psA.tile([P, 512], f32, tag="mm512")
                    for a in range(2):
                        nc.tensor.matmul(
                            ps[:],
                            lhsT=wob[:, a * DIM + mt * P: a * DIM + (mt + 1) * P],
                            rhs=o2[a][:, qb * 512:(qb + 1) * 512],
                            start=(a == 0), stop=(a == 1))
                    yt = yp.tile([P, 512], f32, tag="yt")
                    nc.scalar.copy(yt[:], ps[:])
                    nc.sync.dma_start(
                        d_yt[mt * P:(mt + 1) * P, qb * 512:(qb + 1) * 512],
                        yt[:])


def kernel(x, Wq, bq, Wk, bk, Wv, bv, Wo, bo, src_blocks, tgt_blocks,
           _trace=False):
    global LAST_EXEC_NS, LAST_TRACE
    x = np.asarray(x, np.float32)
    bm = _block_mask(np.asarray(src_blocks), np.asarray(tgt_blocks))
    strips = _plan_strips(bm)
    use_bias = bool(np.any(np.asarray(bq)) or np.any(np.asarray(bk))
                    or np.any(np.asarray(bv)))
    nc = _build_program(strips, use_bias)

    # host-side shard prep
    # W layout for rhs: w[p, kt*1024 + j] = W[j, kt*128 + p]
    def w_rhs(W):
        Wt = np.ascontiguousarray(np.asarray(W, np.float32).T)  # [in, out]
        return np.ascontiguousarray(
            Wt.reshape(8, P, DIM).transpose(1, 0, 2).reshape(P, 8 * DIM))

    wq_h, wk_h, wv_h = w_rhs(Wq), w_rhs(Wk), w_rhs(Wv)
    WoT = np.asarray(Wo, np.float32).T  # [in(=64*head), out]
    x4 = x.reshape(B, NHEADS, P, DIM)

    in_maps = []
    for c in range(NCORES):
        b = c // 4
        h0 = 4 * (c % 4)
        xc = x4[b, h0:h0 + 4]                       # [4, 128, 1024]
        xt = np.ascontiguousarray(xc.transpose(0, 2, 1))  # [4, 1024, 128]
        # xt dram layout [4, 128, 8*128]: xts[i, p, kt*128+t] = x[t, kt*128+p]
        xts = np.ascontiguousarray(
            xt.reshape(HPC, 8, P, P).transpose(0, 2, 1, 3).reshape(HPC, P, 8 * P))
        wo_c = np.zeros((2, P, DIM), ml_dtypes.bfloat16)
        for a in range(2):
            r0 = 64 * (h0 + 2 * a)
            wo_c[a] = WoT[r0:r0 + 128].astype(ml_dtypes.bfloat16)
        in_maps.append({
            "xt": xts,
            "wq": wq_h, "wk": wk_h, "wv": wv_h,
            "bq": np.asarray(bq, np.float32).reshape(1, DIM),
            "bk": np.asarray(bk, np.float32).reshape(1, DIM),
            "bv": np.asarray(bv, np.float32).reshape(1, DIM),
            "wo": wo_c,
        })

    if _trace:
        try:
            import sys
            sys.path.insert(0, "/root/problem/work")
            import ntff_shim
            ntff_shim.install()
        except Exception:
            pass
    res = run_bass_kernel_spmd(nc, in_maps, core_ids=list(range(NCORES)),
                               trace=_trace)
    LAST_EXEC_NS = res.exec_time_ns
    LAST_TRACE = (res.instructions_and_trace[1]
                  if res.instructions_and_trace else None)

    y = np.zeros((B, S, DIM), np.float32)
    for c in range(NCORES):
        y[c // 4] += res.results[c]["yt"].T
    y += np.asarray(bo, np.float32)
    return y
